# revision 1
# baseline (speedup 1.0000x reference)
"""Trainium2 Bass kernel for nn_Net_17532056502451.

5 "think" iterations: shift-window cosine selector (159 shifts) + softmax
attention + scatter-back + conv-style encoder/decoder with energy argmax
(81 shifts), masked-MSE losses averaged.  Data-parallel: 1024 tokens over
8 cores, 128 tokens/core (one per SBUF partition), token-major fp32.

Mappings per core:
- dot correlation: 80 fused scalar_tensor_tensor MACs (DVE).
- sliding norms: Square + prefix-scan + strided diff.
- argmaxes: nc.vector.max / max_index (first-occurrence ties = jnp.argmax).
- per-token dynamic windows: GPSIMD ap_gather (idx[p,j] = off_p + j wrap
  trick, 16 candidate lanes) + 16-way predicated-copy diagonal extract.
- energy: quadratic Gram form. z[t,(d,i)] = ye[t,i]*ye[t,i+d] in ONE DVE op
  (overlapping APs), contracted with host-precomputed A via PE
  transpose -> PSUM->SBUF DMA -> accumulating matmuls.
- encoder/decoder: shared-weight matmuls on yhat (y_att embedded at d*),
  biases folded into PSUM->SBUF activation copies.
"""
import numpy as np

IDIM = 80
ODIM = 80
HDIM = 512
THINK_ITER = 5
TEMPER = 0.7
B, T = 4, 256
NTOK = B * T
P = 128
NCORES = 8
S1 = 159
S2 = 81
NFEAT = 80 * 80
NCHUNK = NFEAT // 128   # 50

_cache = {}


def _build_consts(W_enc, b_enc, W_src, b_src):
    W_enc = np.asarray(W_enc, np.float32)
    b_enc = np.asarray(b_enc, np.float32)
    W_src = np.asarray(W_src, np.float32)
    b_src = np.asarray(b_src, np.float32)
    C = (W_enc.T @ W_enc).astype(np.float32)
    q = (W_enc.T @ b_enc).astype(np.float32)
    bb = np.float32(b_enc @ b_enc)
    # E[t,s] = sum_{d,i} Az[s, 80d+i] * ye_i ye_{i+d} + sum_i 2 q[dd+i] ye_i + bb,
    # dd = 80 - s
    Az = np.zeros((S2, NFEAT), np.float32)
    Al = np.zeros((S2, 81), np.float32)
    for s in range(S2):
        dd = 80 - s
        blk = C[dd:dd + 80, dd:dd + 80]
        for d in range(80):
            diag = np.diagonal(blk, offset=d).copy()
            Az[s, d * 80: d * 80 + (80 - d)] = (2.0 if d > 0 else 1.0) * diag
        Al[s, :80] = 2.0 * q[dd:dd + 80]
        Al[s, 80] = bb
    Az_cat = np.ascontiguousarray(Az.T)               # (6400, 81): pi-major
    Atail = np.ascontiguousarray(Al.T)                # (81, 81)
    W_encT = np.ascontiguousarray(W_enc.T)            # (160, 512)
    W_srcT = np.ascontiguousarray(W_src.T)            # (512, 160)
    M16 = np.zeros((P, 16), np.uint8)
    for p in range(P):
        M16[p, p % 16] = 1
    iota80 = np.broadcast_to(np.arange(80, dtype=np.float32), (P, 80)).copy()
    iota160 = np.broadcast_to(np.arange(160, dtype=np.float32), (P, 160)).copy()
    ident = np.eye(128, dtype=np.float32)
    benc4 = b_enc.reshape(4, 128).T.copy()            # (128, 4)
    bsrc2 = np.zeros((128, 2), np.float32)
    bsrc2[:, 0] = b_src[0:128]
    bsrc2[0:32, 1] = b_src[128:160]
    return dict(Az=Az_cat, Atail=Atail, WencT=W_encT, WsrcT=W_srcT,
                benc=benc4, bsrc=bsrc2, M16=M16, iota80=iota80,
                iota160=iota160, ident=ident,
                ones1=np.ones((1, 128), np.float32))


def _build_nc():
    import concourse.bass as bass
    import concourse.bacc as bacc
    import concourse.mybir as mybir
    from concourse.tile import TileContext

    F32 = mybir.dt.float32
    I16 = mybir.dt.int16
    U32 = mybir.dt.uint32
    Op = mybir.AluOpType
    AF = mybir.ActivationFunctionType

    nc = bacc.Bacc()
    d_x = nc.declare_dram_parameter("x", [P, 80], F32, isOutput=False)
    d_y = nc.declare_dram_parameter("y", [P, 80], F32, isOutput=False)
    d_A = nc.declare_dram_parameter("Az", [NFEAT, 81], F32, isOutput=False)
    d_At = nc.declare_dram_parameter("Atail", [81, 81], F32, isOutput=False)
    d_We = nc.declare_dram_parameter("WencT", [160, 512], F32, isOutput=False)
    d_Ws = nc.declare_dram_parameter("WsrcT", [512, 160], F32, isOutput=False)
    d_be = nc.declare_dram_parameter("benc", [128, 4], F32, isOutput=False)
    d_bs = nc.declare_dram_parameter("bsrc", [128, 2], F32, isOutput=False)
    d_M = nc.declare_dram_parameter("M16", [P, 16], mybir.dt.uint8, isOutput=False)
    d_i80 = nc.declare_dram_parameter("iota80", [P, 80], F32, isOutput=False)
    d_i160 = nc.declare_dram_parameter("iota160", [P, 160], F32, isOutput=False)
    d_id = nc.declare_dram_parameter("ident", [128, 128], F32, isOutput=False)
    d_on = nc.declare_dram_parameter("ones1", [1, 128], F32, isOutput=False)
    d_out = nc.declare_dram_parameter("losspart", [P, 8], F32, isOutput=True)

    with TileContext(nc) as tc:
        with (
            tc.tile_pool(name="const", bufs=1) as cpool,
            tc.tile_pool(name="work", bufs=1) as pool,
            tc.tile_pool(name="zrot", bufs=3) as zpool,
            tc.tile_pool(name="ps_rot", bufs=3, space="PSUM") as pp,
            tc.tile_pool(name="ps_acc", bufs=1, space="PSUM") as ppe,
        ):
            # ---- constants ----
            A_t = cpool.tile([P, NCHUNK * 81], F32, tag="A")
            for k in range(NCHUNK):
                nc.sync.dma_start(A_t[:, k * 81:(k + 1) * 81],
                                  d_A[k * 128:(k + 1) * 128, :])
            At_t = cpool.tile([81, 81], F32, tag="At")
            nc.sync.dma_start(At_t[:], d_At[:])
            We_t = cpool.tile([P, 2 * 512], F32, tag="We")
            nc.sync.dma_start(We_t[:, 0:512], d_We[0:128, :])
            nc.sync.dma_start(We_t[0:32, 512:1024], d_We[128:160, :])
            Ws_t = cpool.tile([P, 4 * 160], F32, tag="Ws")
            for k in range(4):
                nc.sync.dma_start(Ws_t[:, k * 160:(k + 1) * 160],
                                  d_Ws[k * 128:(k + 1) * 128, :])
            be_t = cpool.tile([128, 4], F32, tag="be")
            nc.sync.dma_start(be_t[:], d_be[:])
            bs_t = cpool.tile([128, 2], F32, tag="bs")
            nc.sync.dma_start(bs_t[:], d_bs[:])
            M_t = cpool.tile([P, 16], mybir.dt.uint8, tag="M")
            nc.sync.dma_start(M_t[:], d_M[:])
            i80_t = cpool.tile([P, 80], F32, tag="i80")
            nc.sync.dma_start(i80_t[:], d_i80[:])
            i160_t = cpool.tile([P, 160], F32, tag="i160")
            nc.sync.dma_start(i160_t[:], d_i160[:])
            id_t = cpool.tile([128, 128], F32, tag="id")
            nc.sync.dma_start(id_t[:], d_id[:])

            # ---- state ----
            xpad = pool.tile([P, 238], F32, tag="xpad")
            yres = pool.tile([P, 80], F32, tag="yres")
            keep = pool.tile([P, 80], F32, tag="keep")
            yap = pool.tile([P, 240], F32, tag="yap")
            lossp = pool.tile([P, 8], F32, tag="lossp")
            nc.vector.memset(xpad[:], 0.0)
            nc.vector.memset(yap[:], 0.0)
            nc.vector.memset(lossp[:], 0.0)
            nc.sync.dma_start(xpad[:, 79:159], d_x[:])
            nc.sync.dma_start(yres[:], d_y[:])
            nc.vector.tensor_scalar(keep[:], yres[:], 0.0, None, Op.not_equal)

            sqx = pool.tile([P, 239], F32, tag="sqx")
            nc.vector.memset(sqx[:, 0:1], 0.0)
            cs = pool.tile([P, 239], F32, tag="cs")
            nsq = pool.tile([P, S1], F32, tag="nsq")
            dot = pool.tile([P, S1], F32, tag="dot")
            adot = pool.tile([P, S1], F32, tag="adot")
            gsel = pool.tile([P, S1], F32, tag="gsel")
            rnsq = pool.tile([P, S1], F32, tag="rnsq")
            mx8 = pool.tile([P, 8], F32, tag="mx8")
            mi8 = pool.tile([P, 8], U32, tag="mi8")
            thf = pool.tile([P, 1], F32, tag="thf")
            idxf = pool.tile([P, 160], F32, tag="idxf")
            idxi = pool.tile([P, 160], I16, tag="idxi")
            g1280 = pool.tile([P, 1280], F32, tag="g1280")
            g2560 = pool.tile([P, 2560], F32, tag="g2560")
            yal = pool.tile([P, 80], F32, tag="yal")
            zt = pool.tile([P, 80], F32, tag="zt")
            et = pool.tile([P, 80], F32, tag="et")
            ssum = pool.tile([P, 1], F32, tag="ssum")
            rsum = pool.tile([P, 1], F32, tag="rsum")
            nzm = pool.tile([P, 1], F32, tag="nzm")
            zero1 = pool.tile([P, 1], F32, tag="zero1")
            nc.vector.memset(zero1[:], 0.0)
            xele = pool.tile([P, 80], F32, tag="xele")
            zfeat = pool.tile([P, NFEAT], F32, tag="zfeat")
            e81 = pool.tile([81, 128], F32, tag="e81")
            etail = pool.tile([81, 128], F32, tag="etail")
            nc.sync.dma_start(etail[80:81, :], d_on[:])
            Etok = pool.tile([P, S2], F32, tag="Etok")
            sf = pool.tile([P, 1], F32, tag="sf")
            df = pool.tile([P, 1], F32, tag="df")
            yhat = pool.tile([P, 160], F32, tag="yhat")
            yhT0 = pool.tile([128, 128], F32, tag="yhT0")
            yhT1 = pool.tile([32, 128], F32, tag="yhT1")
            hsT = pool.tile([128, 4 * 128], F32, tag="hsT")
            xeT0 = pool.tile([128, 128], F32, tag="xeT0")
            xeT1 = pool.tile([32, 128], F32, tag="xeT1")
            xext = pool.tile([P, 160], F32, tag="xext")
            yele = pool.tile([P, 80], F32, tag="yele")
            dtmp = pool.tile([P, 80], F32, tag="dtmp")

            ye_view = yap[:, 80:240]

            def gather_extract(src_ap, src_elems, width, out_tile, gbuf):
                """out[p, j] = src[p, idxf[p, j]], j in [0,width)."""
                nc.vector.tensor_copy(idxi[:, 0:width], idxf[:, 0:width])
                nc.gpsimd.ap_gather(gbuf[:, 0:16 * width], src_ap,
                                    idxi[:, 0:width], channels=128,
                                    num_elems=src_elems, d=1,
                                    num_idxs=16 * width)
                gv = gbuf[:, 0:16 * width].rearrange("p (j k) -> p j k", k=16)
                for k in range(16):
                    nc.vector.copy_predicated(
                        out_tile[:, 0:width],
                        M_t[:, k:k + 1].to_broadcast((P, width)),
                        gv[:, :, k])

            for it in range(THINK_ITER):
                # --- sliding norms ---
                nc.scalar.activation(sqx[:, 1:239], xpad[:], AF.Square)
                nc.vector.tensor_tensor_scan(cs[:], sqx[:],
                                             zero1[:].to_broadcast((P, 239)),
                                             0.0, Op.add, Op.bypass)
                nc.vector.tensor_tensor(nsq[:], cs[:, 80:239], cs[:, 0:159],
                                        Op.subtract)
                # --- dot: 80 MACs ---
                nc.vector.tensor_scalar_mul(dot[:], xpad[:, 0:S1], yres[:, 0:1])
                for c in range(1, 80):
                    nc.vector.scalar_tensor_tensor(dot[:], xpad[:, c:c + S1],
                                                   yres[:, c:c + 1], dot[:],
                                                   Op.mult, Op.add)
                # --- theta = argmax dot*|dot|/nsq ---
                nc.scalar.activation(adot[:], dot[:], AF.Abs)
                nc.vector.tensor_scalar_max(rnsq[:], nsq[:], 1e-30)
                nc.vector.reciprocal(rnsq[:], rnsq[:])
                nc.vector.tensor_tensor(gsel[:], dot[:], adot[:], Op.mult)
                nc.vector.tensor_tensor(gsel[:], gsel[:], rnsq[:], Op.mult)
                nc.vector.max(mx8[:], gsel[:])
                nc.vector.max_index(mi8[:], mx8[:], gsel[:])
                nc.vector.tensor_copy(thf[:], mi8[:, 0:1])
                # --- y_align gather ---
                nc.vector.scalar_tensor_tensor(idxf[:, 0:80], i80_t[:],
                                               thf[:, 0:1], i80_t[:],
                                               Op.add, Op.bypass)
                gather_extract(xpad[:], 238, 80, yal, g1280)
                # --- softmax attention -> y_att in yap[:, 80:160] ---
                nc.vector.tensor_tensor(zt[:], yal[:], yres[:], Op.mult)
                nc.vector.max(mx8[:], zt[:])
                nc.vector.tensor_scalar_mul(nzm[:], mx8[:, 0:1], -1.0 / TEMPER)
                nc.scalar.activation(et[:], zt[:], AF.Exp, bias=nzm[:, 0:1],
                                     scale=1.0 / TEMPER)
                nc.vector.tensor_reduce(ssum[:], et[:], mybir.AxisListType.X, Op.add)
                nc.vector.reciprocal(rsum[:], ssum[:])
                nc.vector.tensor_tensor(et[:], et[:], yal[:], Op.mult)
                nc.vector.tensor_scalar_mul(yap[:, 80:160], et[:], rsum[:, 0:1])
                # --- z features: z[p, 80d+i] = ye[i] * ye[i+d] ---
                in0 = ye_view[:, 0:80].unsqueeze(1).to_broadcast((P, 80, 80))
                in1 = bass.AP(ye_view.tensor, ye_view.offset,
                              [list(ye_view.ap[0]), [1, 80], [1, 80]])
                zv = zfeat[:].rearrange("p (d i) -> p d i", i=80)
                nc.vector.tensor_tensor(zv, in0, in1, Op.mult)
                # --- x_ele gather: idx = iota80 + (159 - theta) ---
                nc.vector.tensor_scalar_mul(thf[:], thf[:], -1.0)
                nc.vector.tensor_scalar_add(thf[:], thf[:], 159.0)
                nc.vector.scalar_tensor_tensor(idxf[:, 0:80], i80_t[:],
                                               thf[:, 0:1], i80_t[:],
                                               Op.add, Op.bypass)
                gather_extract(yap[:], 240, 80, xele, g1280)
                nc.vector.tensor_tensor(xpad[:, 79:159], xpad[:, 79:159],
                                        xele[:], Op.subtract)
                # --- E accumulation: pipelined T -> DMA -> MM ---
                Eps = ppe.tile([81, 128], F32, tag="Eps")
                zsb = [None] * NCHUNK
                for k in range(NCHUNK + 2):
                    if k < NCHUNK:
                        zTp = pp.tile([128, 128], F32, tag="zTp")
                        nc.tensor.transpose(zTp[:],
                                            zfeat[:, k * 128:(k + 1) * 128],
                                            id_t[:])
                        zsb_k = zpool.tile([128, 128], F32, tag="zT")
                        zsb[k] = zsb_k
                        nc.scalar.copy(zsb[k][:], zTp[:])
                    j = k - 2
                    if 0 <= j < NCHUNK:
                        nc.tensor.matmul(Eps[:], A_t[:, j * 81:(j + 1) * 81],
                                         zsb[j][:], start=(j == 0), stop=False)
                # tail: feats [ya(80); 1]
                yaTp = pp.tile([128, 128], F32, tag="zTp")
                nc.tensor.transpose(yaTp[0:80, :], yap[:, 80:160], id_t[:])
                nc.scalar.copy(etail[0:80, :], yaTp[0:80, :])
                nc.tensor.matmul(Eps[:], At_t[:], etail[:], start=False,
                                 stop=True)
                # E back to token-major
                nc.scalar.copy(e81[:], Eps[:])
                Etp = pp.tile([128, 128], F32, tag="zTp")
                nc.tensor.transpose(Etp[:, 0:81], e81[:], id_t[0:81, 0:81])
                nc.scalar.copy(Etok[:], Etp[:, 0:81])
                # --- s* argmax, d* = 80 - s* ---
                nc.vector.max(mx8[:], Etok[:])
                nc.vector.max_index(mi8[:], mx8[:], Etok[:])
                nc.vector.tensor_copy(sf[:], mi8[:, 0:1])
                nc.vector.tensor_scalar_mul(df[:], sf[:], -1.0)
                nc.vector.tensor_scalar_add(df[:], df[:], 80.0)
                # --- yhat embed: idx = iota160 + s* ---
                nc.vector.scalar_tensor_tensor(idxf[:, 0:160], i160_t[:],
                                               sf[:, 0:1], i160_t[:],
                                               Op.add, Op.bypass)
                gather_extract(yap[:], 240, 80, yhat, g1280)
                nc.vector.tensor_copy(idxi[:, 0:80], idxf[:, 80:160])
                nc.gpsimd.ap_gather(g1280[:], yap[:], idxi[:, 0:80],
                                    channels=128, num_elems=240, d=1,
                                    num_idxs=1280)
                gv2 = g1280[:].rearrange("p (j k) -> p j k", k=16)
                for k2 in range(16):
                    nc.vector.copy_predicated(
                        yhat[:, 80:160],
                        M_t[:, k2:k2 + 1].to_broadcast((P, 80)),
                        gv2[:, :, k2])
                # --- h_selT = W_enc @ yhat^T (+ b_enc) ---
                yhTp = pp.tile([128, 128], F32, tag="zTp")
                nc.tensor.transpose(yhTp[:], yhat[:, 0:128], id_t[:])
                nc.scalar.copy(yhT0[:], yhTp[:])
                yhTp2 = pp.tile([128, 128], F32, tag="zTp")
                nc.tensor.transpose(yhTp2[0:32, :], yhat[:, 128:160], id_t[:])
                nc.scalar.copy(yhT1[:], yhTp2[0:32, :])
                for hc in range(4):
                    Hp = pp.tile([128, 128], F32, tag="Hp")
                    nc.tensor.matmul(Hp[:], We_t[:, hc * 128:(hc + 1) * 128],
                                     yhT0[:], start=True, stop=False)
                    nc.tensor.matmul(Hp[:],
                                     We_t[0:32, 512 + hc * 128:512 + (hc + 1) * 128],
                                     yhT1[:], start=False, stop=True)
                    nc.scalar.copy(hsT[:, hc * 128:(hc + 1) * 128], Hp[:])
                    nc.vector.tensor_scalar_add(hsT[:, hc * 128:(hc + 1) * 128],
                                                hsT[:, hc * 128:(hc + 1) * 128],
                                                be_t[:, hc:hc + 1])
                # --- x_extT = W_src @ h_selT (+ b_src) ---
                for oc in range(2):
                    ow = 128 if oc == 0 else 32
                    Xp = pp.tile([128, 128], F32, tag="Hp")
                    for hc in range(4):
                        nc.tensor.matmul(
                            Xp[0:ow, :],
                            Ws_t[:, hc * 160 + oc * 128: hc * 160 + oc * 128 + ow],
                            hsT[:, hc * 128:(hc + 1) * 128],
                            start=(hc == 0), stop=(hc == 3))
                    dst = xeT0 if oc == 0 else xeT1
                    nc.scalar.copy(dst[:], Xp[0:ow, :])
                    nc.vector.tensor_scalar_add(dst[:], dst[:],
                                                bs_t[0:ow, oc:oc + 1])
                Xtp = pp.tile([128, 128], F32, tag="Hp")
                nc.tensor.transpose(Xtp[:], xeT0[:], id_t[:])
                nc.scalar.copy(xext[:, 0:128], Xtp[:])
                Xtp2 = pp.tile([128, 128], F32, tag="Hp")
                nc.tensor.transpose(Xtp2[:, 0:32], xeT1[:], id_t[0:32, 0:32])
                nc.scalar.copy(xext[:, 128:160], Xtp2[:, 0:32])
                # --- y_ele gather: idx = iota80 + d* ---
                nc.vector.scalar_tensor_tensor(idxf[:, 0:80], i80_t[:],
                                               df[:, 0:1], i80_t[:],
                                               Op.add, Op.bypass)
                gather_extract(xext[:], 160, 80, yele, g1280)
                # --- loss partial + state updates ---
                nc.vector.tensor_tensor(dtmp[:], yele[:], yres[:], Op.subtract)
                nc.vector.tensor_tensor(dtmp[:], dtmp[:], keep[:], Op.mult)
                nc.vector.tensor_tensor(et[:], dtmp[:], dtmp[:], Op.mult)
                nc.vector.tensor_reduce(lossp[:, it:it + 1], et[:],
                                        mybir.AxisListType.X, Op.add)
                nc.vector.tensor_tensor(yres[:], yres[:], yele[:], Op.subtract)

            nc.sync.dma_start(d_out[:], lossp[:])
    return nc


def kernel(x, y, W_enc, b_enc, W_src, b_src):
    import sys
    if '/opt/trn_rl_repo' not in sys.path:
        sys.path.insert(0, '/opt/trn_rl_repo')
    x = np.asarray(x, np.float32)
    y = np.asarray(y, np.float32)
    consts = _build_consts(W_enc, b_enc, W_src, b_src)

    if "nc" not in _cache:
        _cache["nc"] = _build_nc()
        _cache["nc"].finalize()
    nc = _cache["nc"]

    xt = x.reshape(NTOK, IDIM)
    yt = y.reshape(NTOK, ODIM)
    in_maps = []
    for c in range(NCORES):
        m = dict(consts)
        m["x"] = np.ascontiguousarray(xt[c * P:(c + 1) * P])
        m["y"] = np.ascontiguousarray(yt[c * P:(c + 1) * P])
        in_maps.append(m)

    from concourse.bass_utils import run_bass_kernel_spmd
    res = run_bass_kernel_spmd(nc, in_maps, list(range(NCORES)))
    parts = np.stack([r["losspart"] for r in res.results])
    keep_cnt = max(int((y != 0.0).sum()), 1)
    nums = parts[:, :, :THINK_ITER].sum(axis=(0, 1), dtype=np.float64)
    losses = (nums / keep_cnt).astype(np.float32)
    return np.float32(np.mean(losses))



# revision 4
# speedup vs baseline: 2.8981x; 2.8981x over previous
"""Trainium2 Bass kernel for nn_Net_17532056502451.

5 "think" iterations: shift-window cosine selector (159 shifts) + softmax
attention + scatter-back + conv-style encoder/decoder with energy argmax
(81 shifts), masked-MSE losses averaged.  Data-parallel: 1024 tokens over
8 cores, 128 tokens/core (one per SBUF partition), token-major fp32.

Mappings per core:
- dot correlation: 80 fused scalar_tensor_tensor MACs (DVE).
- sliding norms: Square + prefix-scan + strided diff.
- argmaxes: nc.vector.max / max_index (first-occurrence ties = jnp.argmax).
- per-token dynamic windows: DVE barrel shifter -- log2 stages of in-place
  forward copy_predicated, masks from u32 argmax index via one bitwise_and.
- energy: quadratic Gram form. z[t,(d,i)] = ye[t,i]*ye[t,i+d] in ONE DVE op
  (overlapping APs), contracted with host-precomputed A via PE
  transpose -> PSUM->SBUF DMA -> accumulating matmuls.
- encoder/decoder: shared-weight matmuls on yhat (y_att embedded at d*),
  biases folded into PSUM->SBUF activation copies.
"""
import numpy as np

IDIM = 80
ODIM = 80
HDIM = 512
THINK_ITER = 5
TEMPER = 0.7
B, T = 4, 256
NTOK = B * T
P = 128
NCORES = 8
S1 = 159
S2 = 81
NFEAT = 80 * 80
NCHUNK = NFEAT // 128   # 50

_cache = {}


def _build_consts(W_enc, b_enc, W_src, b_src):
    W_enc = np.asarray(W_enc, np.float32)
    b_enc = np.asarray(b_enc, np.float32)
    W_src = np.asarray(W_src, np.float32)
    b_src = np.asarray(b_src, np.float32)
    C = (W_enc.T @ W_enc).astype(np.float32)
    q = (W_enc.T @ b_enc).astype(np.float32)
    bb = np.float32(b_enc @ b_enc)
    # E[t,s] = sum_{d,i} Az[s, 80d+i] * ye_i ye_{i+d} + sum_i 2 q[dd+i] ye_i + bb,
    # dd = 80 - s
    Az = np.zeros((S2, NFEAT), np.float32)
    Al = np.zeros((S2, 81), np.float32)
    for s in range(S2):
        dd = 80 - s
        blk = C[dd:dd + 80, dd:dd + 80]
        for d in range(80):
            diag = np.diagonal(blk, offset=d).copy()
            Az[s, d * 80: d * 80 + (80 - d)] = (2.0 if d > 0 else 1.0) * diag
        Al[s, :80] = 2.0 * q[dd:dd + 80]
        Al[s, 80] = bb
    Az_cat = np.ascontiguousarray(Az.T)               # (6400, 81): pi-major
    Atail = np.ascontiguousarray(Al.T)                # (81, 81)
    W_encT = np.ascontiguousarray(W_enc.T)            # (160, 512)
    W_srcT = np.ascontiguousarray(W_src.T)            # (512, 160)
    ident = np.eye(128, dtype=np.float32)
    benc4 = b_enc.reshape(4, 128).T.copy()            # (128, 4)
    bsrc2 = np.zeros((128, 2), np.float32)
    bsrc2[:, 0] = b_src[0:128]
    bsrc2[0:32, 1] = b_src[128:160]
    mrow = np.broadcast_to(
        (1 << np.arange(8, dtype=np.uint32)), (P, 8)).copy()  # (128, 8)
    return dict(Az=Az_cat, Atail=Atail, WencT=W_encT, WsrcT=W_srcT,
                benc=benc4, bsrc=bsrc2, mrow=mrow, ident=ident,
                ones1=np.ones((1, 128), np.float32))


def _build_nc():
    import concourse.bass as bass
    import concourse.bacc as bacc
    import concourse.mybir as mybir
    from concourse.tile import TileContext

    F32 = mybir.dt.float32
    U32 = mybir.dt.uint32
    Op = mybir.AluOpType
    AF = mybir.ActivationFunctionType

    nc = bacc.Bacc()
    d_x = nc.declare_dram_parameter("x", [P, 80], F32, isOutput=False)
    d_y = nc.declare_dram_parameter("y", [P, 80], F32, isOutput=False)
    d_A = nc.declare_dram_parameter("Az", [NFEAT, 81], F32, isOutput=False)
    d_At = nc.declare_dram_parameter("Atail", [81, 81], F32, isOutput=False)
    d_We = nc.declare_dram_parameter("WencT", [160, 512], F32, isOutput=False)
    d_Ws = nc.declare_dram_parameter("WsrcT", [512, 160], F32, isOutput=False)
    d_be = nc.declare_dram_parameter("benc", [128, 4], F32, isOutput=False)
    d_bs = nc.declare_dram_parameter("bsrc", [128, 2], F32, isOutput=False)
    d_mr = nc.declare_dram_parameter("mrow", [P, 8], U32, isOutput=False)
    d_id = nc.declare_dram_parameter("ident", [128, 128], F32, isOutput=False)
    d_on = nc.declare_dram_parameter("ones1", [1, 128], F32, isOutput=False)
    d_out = nc.declare_dram_parameter("losspart", [P, 8], F32, isOutput=True)

    with TileContext(nc) as tc:
        with (
            tc.tile_pool(name="const", bufs=1) as cpool,
            tc.tile_pool(name="work", bufs=1) as pool,
            tc.tile_pool(name="zrot", bufs=3) as zpool,
            tc.tile_pool(name="ps_rot", bufs=3, space="PSUM") as pp,
            tc.tile_pool(name="ps_acc", bufs=1, space="PSUM") as ppe,
        ):
            # ---- constants ----
            A_t = cpool.tile([P, NCHUNK * 81], F32, tag="A")
            for k in range(NCHUNK):
                nc.sync.dma_start(A_t[:, k * 81:(k + 1) * 81],
                                  d_A[k * 128:(k + 1) * 128, :])
            At_t = cpool.tile([81, 81], F32, tag="At")
            nc.sync.dma_start(At_t[:], d_At[:])
            We_t = cpool.tile([P, 2 * 512], F32, tag="We")
            nc.sync.dma_start(We_t[:, 0:512], d_We[0:128, :])
            nc.sync.dma_start(We_t[0:32, 512:1024], d_We[128:160, :])
            Ws_t = cpool.tile([P, 4 * 160], F32, tag="Ws")
            for k in range(4):
                nc.sync.dma_start(Ws_t[:, k * 160:(k + 1) * 160],
                                  d_Ws[k * 128:(k + 1) * 128, :])
            be_t = cpool.tile([128, 4], F32, tag="be")
            nc.sync.dma_start(be_t[:], d_be[:])
            bs_t = cpool.tile([128, 2], F32, tag="bs")
            nc.sync.dma_start(bs_t[:], d_bs[:])
            mr_t = cpool.tile([P, 8], U32, tag="mr")
            nc.sync.dma_start(mr_t[:], d_mr[:])
            id_t = cpool.tile([128, 128], F32, tag="id")
            nc.sync.dma_start(id_t[:], d_id[:])

            # ---- state ----
            xpad = pool.tile([P, 335], F32, tag="xpad")
            yres = pool.tile([P, 80], F32, tag="yres")
            keep = pool.tile([P, 80], F32, tag="keep")
            yap = pool.tile([P, 335], F32, tag="yap")
            lossp = pool.tile([P, 8], F32, tag="lossp")
            nc.vector.memset(xpad[:], 0.0)
            nc.vector.memset(yap[:], 0.0)
            nc.vector.memset(lossp[:], 0.0)
            nc.sync.dma_start(xpad[:, 79:159], d_x[:])
            nc.sync.dma_start(yres[:], d_y[:])
            nc.vector.tensor_scalar(keep[:], yres[:], 0.0, None, Op.not_equal)

            sqx = pool.tile([P, 239], F32, tag="sqx")
            nc.vector.memset(sqx[:, 0:1], 0.0)
            cs = pool.tile([P, 239], F32, tag="cs")
            nsq = pool.tile([P, S1], F32, tag="nsq")
            dot = pool.tile([P, S1], F32, tag="dot")
            adot = pool.tile([P, S1], F32, tag="adot")
            gsel = pool.tile([P, S1], F32, tag="gsel")
            rnsq = pool.tile([P, S1], F32, tag="rnsq")
            mx8 = pool.tile([P, 8], F32, tag="mx8")
            mi8 = pool.tile([P, 8], U32, tag="mi8")
            t2 = pool.tile([P, 1], U32, tag="t2")
            d4 = pool.tile([P, 1], U32, tag="d4")
            m8a = pool.tile([P, 8], U32, tag="m8a")
            m8b = pool.tile([P, 8], U32, tag="m8b")
            m8c = pool.tile([P, 8], U32, tag="m8c")
            m8d = pool.tile([P, 8], U32, tag="m8d")
            bbYal = pool.tile([P, 208], F32, tag="bbYal")
            bbXele = pool.tile([P, 208], F32, tag="bbXele")
            bbYhat = pool.tile([P, 224], F32, tag="bbYhat")
            bbYele = pool.tile([P, 144], F32, tag="bbYele")
            yal = bbYal[:, 0:80]
            xele = bbXele[:, 0:80]
            yhat = bbYhat[:, 0:160]
            yele = bbYele[:, 0:80]
            zt = pool.tile([P, 80], F32, tag="zt")
            et = pool.tile([P, 80], F32, tag="et")
            ssum = pool.tile([P, 1], F32, tag="ssum")
            rsum = pool.tile([P, 1], F32, tag="rsum")
            nzm = pool.tile([P, 1], F32, tag="nzm")
            zero1 = pool.tile([P, 1], F32, tag="zero1")
            nc.vector.memset(zero1[:], 0.0)
            zfeat = pool.tile([P, NFEAT], F32, tag="zfeat")
            e81 = pool.tile([81, 128], F32, tag="e81")
            etail = pool.tile([81, 128], F32, tag="etail")
            nc.sync.dma_start(etail[80:81, :], d_on[:])
            Etok = pool.tile([P, S2], F32, tag="Etok")
            yhT0 = pool.tile([128, 128], F32, tag="yhT0")
            yhT1 = pool.tile([32, 128], F32, tag="yhT1")
            hsT = pool.tile([128, 4 * 128], F32, tag="hsT")
            xeT0 = pool.tile([128, 128], F32, tag="xeT0")
            xeT1 = pool.tile([32, 128], F32, tag="xeT1")
            xext = pool.tile([P, 208], F32, tag="xext")
            nc.vector.memset(xext[:, 160:208], 0.0)
            dtmp = pool.tile([P, 80], F32, tag="dtmp")

            ye_view = yap[:, 80:240]

            def barrel(src_pad, m8, buf, out_w, nbits):
                """buf[p, 0:out_w] = src_pad[p, off_p : off_p + out_w] where
                off_p's bit-k mask is m8[:, k] (nonzero when bit set)."""
                k = nbits - 1
                w = out_w + (1 << k) - 1
                nc.scalar.copy(buf[:, 0:w], src_pad[:, 0:w])
                nc.vector.copy_predicated(
                    buf[:, 0:w], m8[:, k:k + 1].to_broadcast((P, w)),
                    src_pad[:, (1 << k):(1 << k) + w])
                for k in range(nbits - 2, -1, -1):
                    w = out_w + (1 << k) - 1
                    nc.vector.copy_predicated(
                        buf[:, 0:w], m8[:, k:k + 1].to_broadcast((P, w)),
                        buf[:, (1 << k):(1 << k) + w])

            for it in range(THINK_ITER):
                # --- sliding norms ---
                nc.scalar.activation(sqx[:, 1:239], xpad[:, 0:238], AF.Square)
                nc.vector.tensor_tensor_scan(cs[:], sqx[:],
                                             zero1[:].to_broadcast((P, 239)),
                                             0.0, Op.add, Op.bypass)
                nc.vector.tensor_tensor(nsq[:], cs[:, 80:239], cs[:, 0:159],
                                        Op.subtract)
                # --- dot: 80 MACs ---
                nc.vector.tensor_scalar_mul(dot[:], xpad[:, 0:S1], yres[:, 0:1])
                for c in range(1, 80):
                    nc.vector.scalar_tensor_tensor(dot[:], xpad[:, c:c + S1],
                                                   yres[:, c:c + 1], dot[:],
                                                   Op.mult, Op.add)
                # --- theta = argmax dot*|dot|/nsq ---
                nc.scalar.activation(adot[:], dot[:], AF.Abs)
                nc.vector.tensor_scalar_max(rnsq[:], nsq[:], 1e-30)
                nc.vector.reciprocal(rnsq[:], rnsq[:])
                nc.vector.tensor_tensor(gsel[:], dot[:], adot[:], Op.mult)
                nc.vector.tensor_tensor(gsel[:], gsel[:], rnsq[:], Op.mult)
                nc.vector.max(mx8[:], gsel[:])
                nc.vector.max_index(mi8[:], mx8[:], gsel[:])
                # --- masks: theta bits, (159 - theta) bits ---
                nc.vector.tensor_tensor(m8a[:], mi8[:, 0:1].to_broadcast((P, 8)),
                                        mr_t[:], Op.bitwise_and)
                nc.vector.tensor_scalar(t2[:], mi8[:, 0:1], 0, None,
                                        Op.bitwise_not)
                nc.vector.tensor_scalar(t2[:], t2[:], 160, None, Op.add)
                nc.vector.tensor_tensor(m8b[:], t2[:].to_broadcast((P, 8)),
                                        mr_t[:], Op.bitwise_and)
                # --- y_align gather: yal[j] = xpad[theta + j] ---
                barrel(xpad, m8a, bbYal, 80, 8)
                # --- softmax attention -> y_att in yap[:, 80:160] ---
                nc.vector.tensor_tensor(zt[:], yal, yres[:], Op.mult)
                nc.vector.max(mx8[:], zt[:])
                nc.vector.tensor_scalar_mul(nzm[:], mx8[:, 0:1], -1.0 / TEMPER)
                nc.scalar.activation(et[:], zt[:], AF.Exp, bias=nzm[:, 0:1],
                                     scale=1.0 / TEMPER)
                nc.vector.tensor_reduce(ssum[:], et[:], mybir.AxisListType.X, Op.add)
                nc.vector.reciprocal(rsum[:], ssum[:])
                nc.vector.tensor_tensor(et[:], et[:], yal, Op.mult)
                nc.vector.tensor_scalar_mul(yap[:, 80:160], et[:], rsum[:, 0:1])
                # --- z features: z[p, 80d+i] = ye[i] * ye[i+d] ---
                in0 = ye_view[:, 0:80].unsqueeze(1).to_broadcast((P, 80, 80))
                in1 = bass.AP(ye_view.tensor, ye_view.offset,
                              [list(ye_view.ap[0]), [1, 80], [1, 80]])
                zv = zfeat[:].rearrange("p (d i) -> p d i", i=80)
                nc.vector.tensor_tensor(zv, in0, in1, Op.mult)
                # --- x_ele gather: xele[j] = yap[(159 - theta) + j] ---
                barrel(yap, m8b, bbXele, 80, 8)
                nc.vector.tensor_tensor(xpad[:, 79:159], xpad[:, 79:159],
                                        xele, Op.subtract)
                # --- E accumulation: pipelined T -> copy -> MM ---
                Eps = ppe.tile([81, 128], F32, tag="Eps")
                zsb = [None] * NCHUNK
                for k in range(NCHUNK + 2):
                    if k < NCHUNK:
                        zTp = pp.tile([128, 128], F32, tag="zTp")
                        nc.tensor.transpose(zTp[:],
                                            zfeat[:, k * 128:(k + 1) * 128],
                                            id_t[:])
                        zsb_k = zpool.tile([128, 128], F32, tag="zT")
                        zsb[k] = zsb_k
                        nc.scalar.copy(zsb[k][:], zTp[:])
                    j = k - 2
                    if 0 <= j < NCHUNK:
                        nc.tensor.matmul(Eps[:], A_t[:, j * 81:(j + 1) * 81],
                                         zsb[j][:], start=(j == 0), stop=False)
                # tail: feats [ya(80); 1]
                yaTp = pp.tile([128, 128], F32, tag="zTp")
                nc.tensor.transpose(yaTp[0:80, :], yap[:, 80:160], id_t[:])
                nc.scalar.copy(etail[0:80, :], yaTp[0:80, :])
                nc.tensor.matmul(Eps[:], At_t[:], etail[:], start=False,
                                 stop=True)
                # E back to token-major
                nc.scalar.copy(e81[:], Eps[:])
                Etp = pp.tile([128, 128], F32, tag="zTp")
                nc.tensor.transpose(Etp[:, 0:81], e81[:], id_t[0:81, 0:81])
                nc.scalar.copy(Etok[:], Etp[:, 0:81])
                # --- s* argmax; masks for s* and d* = 80 - s* ---
                nc.vector.max(mx8[:], Etok[:])
                nc.vector.max_index(mi8[:], mx8[:], Etok[:])
                nc.vector.tensor_tensor(m8c[:], mi8[:, 0:1].to_broadcast((P, 8)),
                                        mr_t[:], Op.bitwise_and)
                nc.vector.tensor_scalar(d4[:], mi8[:, 0:1], 0, None,
                                        Op.bitwise_not)
                nc.vector.tensor_scalar(d4[:], d4[:], 81, None, Op.add)
                nc.vector.tensor_tensor(m8d[:], d4[:].to_broadcast((P, 8)),
                                        mr_t[:], Op.bitwise_and)
                # --- yhat embed: yhat[j] = yap[s* + j], j in [0,160) ---
                barrel(yap, m8c, bbYhat, 160, 7)
                # --- h_selT = W_enc @ yhat^T (+ b_enc) ---
                yhTp = pp.tile([128, 128], F32, tag="zTp")
                nc.tensor.transpose(yhTp[:], bbYhat[:, 0:128], id_t[:])
                nc.scalar.copy(yhT0[:], yhTp[:])
                yhTp2 = pp.tile([128, 128], F32, tag="zTp")
                nc.tensor.transpose(yhTp2[0:32, :], bbYhat[:, 128:160], id_t[:])
                nc.scalar.copy(yhT1[:], yhTp2[0:32, :])
                for hc in range(4):
                    Hp = pp.tile([128, 128], F32, tag="Hp")
                    nc.tensor.matmul(Hp[:], We_t[:, hc * 128:(hc + 1) * 128],
                                     yhT0[:], start=True, stop=False)
                    nc.tensor.matmul(Hp[:],
                                     We_t[0:32, 512 + hc * 128:512 + (hc + 1) * 128],
                                     yhT1[:], start=False, stop=True)
                    nc.scalar.copy(hsT[:, hc * 128:(hc + 1) * 128], Hp[:])
                    nc.vector.tensor_scalar_add(hsT[:, hc * 128:(hc + 1) * 128],
                                                hsT[:, hc * 128:(hc + 1) * 128],
                                                be_t[:, hc:hc + 1])
                # --- x_extT = W_src @ h_selT (+ b_src) ---
                for oc in range(2):
                    ow = 128 if oc == 0 else 32
                    Xp = pp.tile([128, 128], F32, tag="Hp")
                    for hc in range(4):
                        nc.tensor.matmul(
                            Xp[0:ow, :],
                            Ws_t[:, hc * 160 + oc * 128: hc * 160 + oc * 128 + ow],
                            hsT[:, hc * 128:(hc + 1) * 128],
                            start=(hc == 0), stop=(hc == 3))
                    dst = xeT0 if oc == 0 else xeT1
                    nc.scalar.copy(dst[:], Xp[0:ow, :])
                    nc.vector.tensor_scalar_add(dst[:], dst[:],
                                                bs_t[0:ow, oc:oc + 1])
                Xtp = pp.tile([128, 128], F32, tag="Hp")
                nc.tensor.transpose(Xtp[:], xeT0[:], id_t[:])
                nc.scalar.copy(xext[:, 0:128], Xtp[:])
                Xtp2 = pp.tile([128, 128], F32, tag="Hp")
                nc.tensor.transpose(Xtp2[:, 0:32], xeT1[:], id_t[0:32, 0:32])
                nc.scalar.copy(xext[:, 128:160], Xtp2[:, 0:32])
                # --- y_ele gather: yele[j] = xext[d* + j] ---
                barrel(xext, m8d, bbYele, 80, 7)
                # --- loss partial + state updates ---
                nc.vector.tensor_tensor(dtmp[:], yele, yres[:], Op.subtract)
                nc.vector.tensor_tensor(dtmp[:], dtmp[:], keep[:], Op.mult)
                nc.vector.tensor_tensor(et[:], dtmp[:], dtmp[:], Op.mult)
                nc.vector.tensor_reduce(lossp[:, it:it + 1], et[:],
                                        mybir.AxisListType.X, Op.add)
                nc.vector.tensor_tensor(yres[:], yres[:], yele, Op.subtract)

            nc.sync.dma_start(d_out[:], lossp[:])
    return nc


def kernel(x, y, W_enc, b_enc, W_src, b_src):
    import sys
    if '/opt/trn_rl_repo' not in sys.path:
        sys.path.insert(0, '/opt/trn_rl_repo')
    x = np.asarray(x, np.float32)
    y = np.asarray(y, np.float32)
    consts = _build_consts(W_enc, b_enc, W_src, b_src)

    if "nc" not in _cache:
        _cache["nc"] = _build_nc()
        _cache["nc"].finalize()
    nc = _cache["nc"]

    xt = x.reshape(NTOK, IDIM)
    yt = y.reshape(NTOK, ODIM)
    in_maps = []
    for c in range(NCORES):
        m = dict(consts)
        m["x"] = np.ascontiguousarray(xt[c * P:(c + 1) * P])
        m["y"] = np.ascontiguousarray(yt[c * P:(c + 1) * P])
        in_maps.append(m)

    from concourse.bass_utils import run_bass_kernel_spmd
    res = run_bass_kernel_spmd(nc, in_maps, list(range(NCORES)))
    parts = np.stack([r["losspart"] for r in res.results])
    keep_cnt = max(int((y != 0.0).sum()), 1)
    nums = parts[:, :, :THINK_ITER].sum(axis=(0, 1), dtype=np.float64)
    losses = (nums / keep_cnt).astype(np.float32)
    return np.float32(np.mean(losses))
